# revision 18
# baseline (speedup 1.0000x reference)
"""RWKV-4 block (time-mix + channel-mix) Trainium2 Bass kernel.

Sharding: data-parallel over B across 8 NeuronCores (B=8 -> 1 batch el/core,
zero collectives). Everything on-device is feature-major [C partitions, T free]:
 - time_shift == free-dim offset (no data movement)
 - WKV recurrence == native DVE tensor_tensor_scan (state = d0*state + d1)
 - LN stats via ones-matmul partition reduction + K=1 ones-matmul broadcast
 - LN gain/bias folded into the GEMM weights host-side
GEMMs run as fp32r (full-rate fp32 mode). Channel-mix hidden (relu^2 out) is
spilled to DRAM in bf16 and the Wcv GEMM runs in bf16.
"""

import contextlib

import numpy as np
import ml_dtypes

import concourse.bass as bass
import concourse.bacc as bacc
import concourse.mybir as mybir
import concourse.tile as tile

F32 = mybir.dt.float32
F32R = mybir.dt.float32r
BF16 = mybir.dt.bfloat16
AF = mybir.ActivationFunctionType
ALU = mybir.AluOpType
P = 128
EPS = 1e-5


F8 = mybir.dt.float8e4
DR = mybir.MatmulPerfMode.DoubleRow
S_XK2 = 16.0   # xk2 -> fp8 activation scale
S_KKR = 4.0    # kkr carries 4*relu(k1); kk8 = kkr^2 = 16*kk


class Cfg:
    def __init__(self, B=8, T=2048, C=1024, HID=4096,
                 TS_A=256, TS_C=512, TS_D=512, fp32r=True, fp8_cd=True):
        self.B, self.T, self.C, self.HID = B, T, C, HID
        self.TS_A, self.TS_C, self.TS_D = TS_A, TS_C, TS_D
        self.fp32r = fp32r
        self.fp8_cd = fp8_cd
        assert C % P == 0 and HID % P == 0
        assert T % TS_A == 0 and T % TS_C == 0 and T % TS_D == 0


def _dma_split(nc, out_tile, in_3d, n):
    """Per-subtile DMA: out_tile [P, n, W] <- in_3d [P, n, W] (one dma per mid idx).
    Keeps each PE consumer waiting on a single DMA queue sem."""
    for i in range(n):
        nc.sync.dma_start(out=out_tile[:, i, :], in_=in_3d[:, i, :])


def _f(ap):
    """View an f32r AP as plain f32 for non-matmul engines."""
    return ap.bitcast(F32) if ap.dtype == F32R else ap


def emit(tc, outs, ins, cfg):
    nc = tc.nc
    C, T, HID = cfg.C, cfg.T, cfg.HID
    CT, HT = C // P, HID // P
    TSA, TSC, TSD = cfg.TS_A, cfg.TS_C, cfg.TS_D
    NSA, NSC, NSD = T // TSA, T // TSC, T // TSD

    out_full = outs["out"]          # [C, T] f32
    xT = ins["xT"]                  # [C, T] f32
    MMF = F32R if cfg.fp32r else F32

    ctx = contextlib.ExitStack()
    with ctx:
        constp = ctx.enter_context(tc.tile_pool(name="constp", bufs=1))
        dram = ctx.enter_context(tc.tile_pool(name="dram", bufs=1, space="DRAM"))

        # ---- DRAM scratch ----
        rwkv_d = dram.tile([C, T], BF16, name="rwkv_d")
        x2_d = dram.tile([C, T], MMF, name="x2_d")
        sr2_d = dram.tile([C, T], BF16, name="sr2_d")
        xk2_d = dram.tile([C, T], F8 if cfg.fp8_cd else BF16, name="xk2_d")

        # ---- constants ----
        def load_vec(key, n):
            t = constp.tile([P, n // P], F32, name=f"cv_{key}")
            nc.sync.dma_start(out=t, in_=ins[key].rearrange("(o p) -> p o", p=P))
            return t

        tmk_t = load_vec("tmk", C)
        tmv_t = load_vec("tmv", C)
        tmr_t = load_vec("tmr", C)
        cmk_t = load_vec("cmk", C)
        cmr_t = load_vec("cmr", C)
        eu_t = load_vec("eu", C)
        ew_t = load_vec("ew", C)
        ck_t = load_vec("ck", C)
        cv_t = load_vec("cv", C)
        cr_t = load_vec("cr", C)
        ccr_t = load_vec("ccr", C)
        cck_t = load_vec("cck", HID)
        s01_t = load_vec("s01", C)
        s02_t = load_vec("s02", C)

        ones_t = constp.tile([P, 1], MMF, name="ones_t")
        nc.sync.dma_start(out=ones_t, in_=ins["one"].rearrange("(p a) -> p a", a=1))
        ones_r = constp.tile([1, P], MMF, name="ones_r")
        nc.sync.dma_start(out=ones_r, in_=ins["one"].rearrange("(a p) -> a p", a=1))
        eps_t = constp.tile([P, 1], F32, name="eps_t")
        nc.vector.memset(eps_t, EPS)


        def ln_pre(work, psS, xs, TS, n_ct, tag_sq, tag_bufs=1):
            """Stats for LN over partitions: returns mr2 [1,2,TS] = (mean, rstd)."""
            sq = work.tile([P, n_ct, TS], MMF, name="ln_sq", tag=tag_sq, bufs=tag_bufs)
            nc.scalar.square(sq, _f(xs))
            ps_sum = psS.tile([1, TS], F32, name="ps_sum", tag="ps_sum")
            ps_sq = psS.tile([1, TS], F32, name="ps_sq", tag="ps_sq")
            for ct in range(n_ct):
                nc.tensor.matmul(ps_sum, lhsT=ones_t, rhs=xs[:, ct, :],
                                 start=(ct == 0), stop=(ct == n_ct - 1))
            for ct in range(n_ct):
                nc.tensor.matmul(ps_sq, lhsT=ones_t, rhs=sq[:, ct, :],
                                 start=(ct == 0), stop=(ct == n_ct - 1))
            mr2 = work.tile([1, 2, TS], MMF, name="ln_mr2", tag="ln_mr2", bufs=2)
            mrow = mr2[:, 0, :]
            with nc.allow_low_precision(reason="f32r mean for broadcast matmul"):
                nc.vector.tensor_scalar_mul(mrow, ps_sum, 1.0 / C)
            msq = work.tile([1, TS], F32, name="ln_msq", tag="ln_msq")
            nc.scalar.mul(msq, ps_sq, 1.0 / C)
            var = work.tile([1, TS], F32, name="ln_var", tag="ln_var")
            nc.vector.tensor_mul(var, _f(mrow), _f(mrow))
            nc.vector.tensor_sub(var, msq, var)
            sd = work.tile([1, TS], F32, name="ln_sd", tag="ln_sd")
            nc.scalar.activation(sd, var, AF.Sqrt, bias=eps_t[:1, :])
            with nc.allow_low_precision(reason="f32r rstd for broadcast matmul"):
                nc.vector.reciprocal(mr2[:, 1, :], sd)
            return mr2

        def ln_bc(work, psS, mr2, TS):
            """Broadcast (mean, rstd) to 128 partitions via K=1 ones-matmul."""
            bc_ps = psS.tile([P, 2, TS], F32, name="ln_bcps", tag="ln_bcps")
            nc.tensor.matmul(bc_ps.rearrange("p a t -> p (a t)"),
                             lhsT=ones_r,
                             rhs=mr2.rearrange("p a t -> p (a t)"),
                             start=True, stop=True)
            mbc2 = work.tile([P, 2, TS], F32, name="ln_mbc2", tag="ln_mbc2", bufs=2)
            nc.scalar.copy(mbc2, bc_ps)
            return mbc2[:, 0, :], mbc2[:, 1, :]

        # ============== stages A+B share the carried-state pool ==============
        with tc.tile_pool(name="persAB", bufs=1) as persAB:
            # decay broadcast tiles for the scans (same every slice)
            ewb = persAB.tile([P, CT, TSA], F32, name="ewb")
            for ct in range(CT):
                nc.gpsimd.tensor_copy(
                    out=ewb[:, ct, :],
                    in_=ew_t[:, ct:ct + 1].to_broadcast((P, TSA)),
                )
            eub = persAB.tile([P, CT, TSA], BF16, name="eub")
            for ct in range(CT):
                nc.gpsimd.tensor_copy(
                    out=eub[:, ct, :],
                    in_=eu_t[:, ct:ct + 1].to_broadcast((P, TSA)),
                )
            hbuf = persAB.tile([P, CT, TSA + 1], BF16, name="hbuf")
            nc.vector.tensor_copy(out=hbuf[:, :, 0], in_=s01_t)
            stA = persAB.tile([P, CT, 1], F32, name="stA")
            stB = persAB.tile([P, CT, 1], F32, name="stB")
            nc.vector.memset(stA, 0.0)
            nc.vector.memset(stB, 0.0)

            # ================= stage A: time-mix =================
            with tc.tile_pool(name="wA", bufs=1) as wA, \
                 tc.tile_pool(name="sA_strm", bufs=2) as strm, \
                 tc.tile_pool(name="sA_work", bufs=1) as work, \
                 tc.tile_pool(name="sA_ps", bufs=5, space="PSUM") as psA, \
                 tc.tile_pool(name="sA_psS", bufs=1, space="PSUM") as psS:

                wk_sb = wA.tile([P, CT, C], BF16, name="wk_sb")
                wv_sb = wA.tile([P, CT, C], BF16, name="wv_sb")
                wr_sb = wA.tile([P, CT, C], BF16, name="wr_sb")

                def _gemm_a(wsb, mixj, dst, act, cst):
                    for ot in range(CT):
                        ps = psA.tile([P, TSA], F32, name="gps", tag="gps")
                        for ct in range(CT):
                            nc.tensor.matmul(
                                ps, lhsT=wsb[:, ct, ot * P:(ot + 1) * P],
                                rhs=mixj[:, ct, :],
                                start=(ct == 0), stop=(ct == CT - 1))
                        nc.scalar.activation(dst[:, ot, :], ps, act,
                                             bias=cst[:, ot:ot + 1])

                prev = None
                for s in range(NSA + 1):
                    if s < NSA:
                        t0 = s * TSA
                        xs = strm.tile([P, CT, TSA], MMF, name="xs", tag="xs")
                        _dma_split(nc, xs, xT[:, t0:t0 + TSA]
                                   .rearrange("(ct p) t -> p ct t", p=P), CT)
                        mr2 = ln_pre(work, psS, xs, TSA, CT, "scrA", 2)
                        if s == 0:
                            _dma_split(nc, wk_sb, ins["wk"]
                                       .rearrange("(ct p) o -> p ct o", p=P), CT)
                            _dma_split(nc, wv_sb, ins["wv"]
                                       .rearrange("(ct p) o -> p ct o", p=P), CT)
                            _dma_split(nc, wr_sb, ins["wr"]
                                       .rearrange("(ct p) o -> p ct o", p=P), CT)
                    if prev is not None:
                        Kt = work.tile([P, CT, TSA], BF16, name="Kt", tag="Kt",
                                       bufs=2)
                        _gemm_a(wk_sb, prev["mix3"][:, 0], Kt, AF.Exp, ck_t)
                    if s < NSA:
                        m_bc, rs_bc = ln_bc(work, psS, mr2, TSA)
                    if prev is not None:
                        Vt = work.tile([P, CT, TSA], BF16, name="Vt", tag="Vt",
                                       bufs=2)
                        _gemm_a(wv_sb, prev["mix3"][:, 1], Vt, AF.Identity, cv_t)
                        SRt = strm.tile([P, CT, TSA], BF16, name="SRt", tag="SRt",
                                        bufs=2)
                        _gemm_a(wr_sb, prev["mix3"][:, 2], SRt, AF.Sigmoid, cr_t)

                        KV = work.tile([P, CT, TSA], BF16, name="KV", tag="KV",
                                       bufs=2)
                        nc.gpsimd.tensor_mul(KV, Kt, Vt)
                        # scan outputs carry the inbound state in column 0
                        SA = work.tile([P, CT, TSA + 1], F32, name="SA", tag="SA")
                        SB = work.tile([P, CT, TSA + 1], F32, name="SB", tag="scrA",
                                       bufs=2)
                        nc.vector.tensor_copy(out=SA[:, :, 0:1], in_=stA)
                        nc.vector.tensor_copy(out=SB[:, :, 0:1], in_=stB)
                        for ct in range(CT):
                            nc.vector.tensor_tensor_scan(
                                out=SA[:, ct, 1:], data0=ewb[:, ct, :],
                                data1=KV[:, ct, :],
                                initial=stA[:, ct, :], op0=ALU.mult, op1=ALU.add)
                        for ct in range(CT):
                            nc.vector.tensor_tensor_scan(
                                out=SB[:, ct, 1:], data0=ewb[:, ct, :],
                                data1=Kt[:, ct, :],
                                initial=stB[:, ct, :], op0=ALU.mult, op1=ALU.add)
                        # N into KV, D into Kt (in place)
                        Dt = work.tile([P, CT, TSA], F32, name="Dt", tag="rec")
                        t1 = work.tile([P, CT, TSA], BF16, name="t1", tag="t1",
                                       bufs=2)
                        nc.gpsimd.tensor_mul(t1, KV, eub)
                        nc.gpsimd.tensor_add(KV, t1, SA[:, :, :TSA])
                        t2 = work.tile([P, CT, TSA], BF16, name="t2", tag="t1",
                                       bufs=2)
                        nc.gpsimd.tensor_mul(t2, Kt, eub)
                        nc.gpsimd.tensor_add(Dt, t2, SB[:, :, :TSA])
                        nc.vector.tensor_copy(out=stA, in_=SA[:, :, TSA:TSA + 1])
                        nc.vector.tensor_copy(out=stB, in_=SB[:, :, TSA:TSA + 1])
                        # y = N / D ; rwkv = y * sigmoid(r) -> bf16 for the spill
                        rec = Dt
                        nc.vector.reciprocal_approx_fast(rec, Dt)
                        rwkv16 = work.tile([P, CT, TSA], BF16, name="rwkv16",
                                           tag="dmixA", bufs=2)
                        nc.gpsimd.tensor_mul(KV, KV, rec)
                        nc.gpsimd.tensor_mul(rwkv16, KV, SRt)
                        nc.sync.dma_start(
                            out=rwkv_d[:, prev["t0"]:prev["t0"] + TSA]
                            .rearrange("(ct p) t -> p ct t", p=P),
                            in_=rwkv16)
                        prev = None
                    if s < NSA:
                        if s > 0:
                            nc.vector.tensor_copy(out=hbuf[:, :, 0:1],
                                                  in_=hbuf[:, :, TSA:TSA + 1])
                        d = work.tile([P, CT, TSA], BF16, name="dmix", tag="dmixA",
                                      bufs=2)
                        for ct in range(CT):
                            xc = work.tile([P, TSA], F32, name="xc", tag="xcA",
                                           bufs=2)
                            nc.vector.tensor_sub(xc, _f(xs[:, ct, :]), m_bc)
                            nc.vector.tensor_mul(hbuf[:, ct, 1:], xc, rs_bc)
                        nc.gpsimd.tensor_sub(d, hbuf[:, :, 1:],
                                             hbuf[:, :, :TSA])
                        mix3 = strm.tile([P, 3, CT, TSA], BF16, name="mix3",
                                         tag="mix3")
                        for j, tmx in enumerate((tmk_t, tmv_t, tmr_t)):
                            for ct in range(CT):
                                nc.vector.scalar_tensor_tensor(
                                    out=mix3[:, j, ct, :], in0=d[:, ct, :],
                                    scalar=tmx[:, ct:ct + 1],
                                    in1=hbuf[:, ct, :TSA],
                                    op0=ALU.mult, op1=ALU.add)
                        prev = {"mix3": mix3, "t0": t0}

            # ======== stage B: Wo + residual, LN2, xk2 spill, Wcr ========
            hbuf2 = persAB.tile([P, CT, TSA + 1], BF16, name="hbuf2")
            nc.vector.tensor_copy(out=hbuf2[:, :, 0], in_=s02_t)

            with tc.tile_pool(name="wB", bufs=1) as wB, \
                 tc.tile_pool(name="sB_strm", bufs=2) as strm, \
                 tc.tile_pool(name="sB_work", bufs=1) as work, \
                 tc.tile_pool(name="sB_ps", bufs=5, space="PSUM") as psB, \
                 tc.tile_pool(name="sB_psS", bufs=1, space="PSUM") as psS:

                wo_sb = wB.tile([P, CT, C], BF16, name="wo_sb")
                wcr_sb = wB.tile([P, CT, C], BF16, name="wcr_sb")

                def _gemm_b(wsb, rhs3, post):
                    for ot in range(CT):
                        ps = psB.tile([P, TSA], F32, name="gpsB", tag="gpsB")
                        for ct in range(CT):
                            nc.tensor.matmul(
                                ps, lhsT=wsb[:, ct, ot * P:(ot + 1) * P],
                                rhs=rhs3[:, ct, :],
                                start=(ct == 0), stop=(ct == CT - 1))
                        post(ot, ps)

                from collections import deque
                pend = deque()
                for s in range(NSA + 2):
                    if s < NSA:
                        t0 = s * TSA
                        rw = strm.tile([P, CT, TSA], BF16, name="rw", tag="rw")
                        _dma_split(nc, rw, rwkv_d[:, t0:t0 + TSA]
                                   .rearrange("(ct p) t -> p ct t", p=P), CT)
                        xs = strm.tile([P, CT, TSA], MMF, name="xsB", tag="xsB")
                        nc.sync.dma_start(out=xs, in_=xT[:, t0:t0 + TSA]
                                          .rearrange("(ct p) t -> p ct t", p=P))
                        if s == 0:
                            _dma_split(nc, wo_sb, ins["wo"]
                                       .rearrange("(ct p) o -> p ct o", p=P), CT)
                            _dma_split(nc, wcr_sb, ins["wcr"]
                                       .rearrange("(ct p) o -> p ct o", p=P), CT)
                        x2s = strm.tile([P, CT, TSA], MMF, name="x2s", tag="x2s")
                        _gemm_b(wo_sb, rw, lambda ot, ps: nc.vector.tensor_add(
                            x2s[:, ot, :], ps, _f(xs[:, ot, :])))
                        nc.sync.dma_start(
                            out=x2_d[:, t0:t0 + TSA]
                            .rearrange("(ct p) t -> p ct t", p=P),
                            in_=x2s)
                    prevb = pend.popleft() if (len(pend) >= 2 or s >= NSA) and pend else None
                    if prevb is not None:
                        sr2s = strm.tile([P, CT, TSA], BF16, name="sr2s", tag="sr2s")
                        _gemm_b(wcr_sb, prevb["mixr"],
                                lambda ot, ps: nc.scalar.activation(
                                    sr2s[:, ot, :], ps, AF.Sigmoid,
                                    bias=ccr_t[:, ot:ot + 1]))
                        nc.sync.dma_start(
                            out=sr2_d[:, prevb["t0"]:prevb["t0"] + TSA]
                            .rearrange("(ct p) t -> p ct t", p=P),
                            in_=sr2s)
                    if s < NSA:
                        mr2 = ln_pre(work, psS, x2s, TSA, CT, "sqB")
                        m_bc, rs_bc = ln_bc(work, psS, mr2, TSA)
                        if s > 0:
                            nc.vector.tensor_copy(out=hbuf2[:, :, 0:1],
                                                  in_=hbuf2[:, :, TSA:TSA + 1])
                        d2 = work.tile([P, CT, TSA], BF16, name="d2", tag="d2",
                                       bufs=2)
                        for ct in range(CT):
                            tmp = work.tile([P, TSA], F32, name="tmpB", tag="tmpB",
                                            bufs=2)
                            nc.vector.tensor_sub(tmp, _f(x2s[:, ct, :]), m_bc)
                            nc.vector.tensor_mul(hbuf2[:, ct, 1:], tmp, rs_bc)
                        nc.gpsimd.tensor_sub(d2, hbuf2[:, :, 1:],
                                             hbuf2[:, :, :TSA])
                        xk2s = work.tile([P, CT, TSA], BF16, name="xk2s",
                                         tag="xk2s", bufs=2)
                        for ct in range(CT):
                            nc.vector.scalar_tensor_tensor(
                                out=xk2s[:, ct, :], in0=d2[:, ct, :],
                                scalar=cmk_t[:, ct:ct + 1], in1=hbuf2[:, ct, :TSA],
                                op0=ALU.mult, op1=ALU.add)
                        if cfg.fp8_cd:
                            xk2q = work.tile([P, CT, TSA], F8, name="xk2q",
                                             tag="xk2q", bufs=2)
                            nc.scalar.activation(xk2q, xk2s, AF.Identity,
                                                 scale=S_XK2)
                            nc.sync.dma_start(
                                out=xk2_d[:, t0:t0 + TSA]
                                .rearrange("(ct p) t -> p ct t", p=P),
                                in_=xk2q)
                        else:
                            nc.sync.dma_start(
                                out=xk2_d[:, t0:t0 + TSA]
                                .rearrange("(ct p) t -> p ct t", p=P),
                                in_=xk2s)
                        mixr = work.tile([P, CT, TSA], BF16, name="mixr",
                                         tag="mixr", bufs=3)
                        for ct in range(CT):
                            nc.vector.scalar_tensor_tensor(
                                out=mixr[:, ct, :], in0=d2[:, ct, :],
                                scalar=cmr_t[:, ct:ct + 1], in1=hbuf2[:, ct, :TSA],
                                op0=ALU.mult, op1=ALU.add)
                        pend.append({"mixr": mixr, "t0": t0})

        # ============ stage CD: Wck + relu^2 (SBUF) + Wcv + gate + residual ============
        if cfg.fp8_cd:
            with tc.tile_pool(name="wCD", bufs=1) as wCD, \
                 tc.tile_pool(name="sCD_strm", bufs=1) as strm, \
                 tc.tile_pool(name="sCD_kk", bufs=2) as kkp, \
                 tc.tile_pool(name="sCD_work", bufs=2) as work, \
                 tc.tile_pool(name="sCD_ps1", bufs=2, space="PSUM") as psC, \
                 tc.tile_pool(name="sCD_ps2", bufs=2, space="PSUM") as psD:
                wck_sb = wCD.tile([P, CT, HID], F8, name="wck_sb")
                _dma_split(nc, wck_sb,
                           ins["wck"].rearrange("(ct p) h -> p ct h", p=P), CT)
                wcv_sb = wCD.tile([P, HT, C], F8, name="wcv_sb")
                sck_t = wCD.tile([P, 1], F32, name="sck_t")
                nc.sync.dma_start(out=sck_t,
                                  in_=ins["sck_act"].rearrange("(p a) -> p a", a=1))
                scv_t = wCD.tile([P, 1], F32, name="scv_t")
                nc.sync.dma_start(out=scv_t,
                                  in_=ins["scv_gate"].rearrange("(p a) -> p a", a=1))
                for s in range(NSD):
                    t0 = s * TSD
                    xk2s = strm.tile([P, CT, TSD], F8, name="xk2sC",
                                     tag="xk2sC", bufs=2)
                    _dma_split(nc, xk2s, xk2_d[:, t0:t0 + TSD]
                               .rearrange("(ct p) t -> p ct t", p=P), CT)
                    if s == 0:
                        _dma_split(nc, wcv_sb, ins["wcv"]
                                   .rearrange("(ht p) o -> p ht o", p=P), HT)
                    kk8 = kkp.tile([P, HT, TSD], F8, name="kk8", tag="kk8")
                    for hh in range(HT // 2):
                        ps = psC.tile([P, 2, TSD], F32, name="gpsC", tag="gpsC")
                        for i in range(2):
                            ht = 2 * hh + i
                            for j in range(CT // 2):
                                nc.tensor.matmul(
                                    ps[:, i, :],
                                    lhsT=wck_sb[:, 2 * j:2 * j + 2,
                                                ht * P:(ht + 1) * P],
                                    rhs=xk2s[:, 2 * j:2 * j + 2, :],
                                    start=(j == 0), stop=(j == CT // 2 - 1),
                                    perf_mode=DR)
                        kkr = work.tile([P, 2, TSD], BF16, name="kkr", tag="kkr")
                        for i in range(2):
                            ht = 2 * hh + i
                            nc.scalar.activation(kkr[:, i, :], ps[:, i, :],
                                                 AF.Relu, scale=sck_t[:, :],
                                                 bias=cck_t[:, ht:ht + 1])
                        nc.scalar.activation(kk8[:, 2 * hh:2 * hh + 2, :], kkr,
                                             AF.Square)
                    sr2s = strm.tile([P, CT, TSD], BF16, name="sr2sD",
                                     tag="sr2sD", bufs=1)
                    _dma_split(nc, sr2s, sr2_d[:, t0:t0 + TSD]
                               .rearrange("(ct p) t -> p ct t", p=P), CT)
                    x2s = strm.tile([P, CT, TSD], MMF, name="x2sD",
                                    tag="x2sD", bufs=1)
                    _dma_split(nc, x2s, x2_d[:, t0:t0 + TSD]
                               .rearrange("(ct p) t -> p ct t", p=P), CT)
                    ot_out = strm.tile([P, CT, TSD], F32, name="ot_out",
                                       tag="ot_out", bufs=1)
                    for ot in range(CT):
                        ps = psD.tile([P, TSD], F32, name="gpsD", tag="gpsD")
                        for j in range(HT // 2):
                            nc.tensor.matmul(
                                ps,
                                lhsT=wcv_sb[:, 2 * j:2 * j + 2,
                                            ot * P:(ot + 1) * P],
                                rhs=kk8[:, 2 * j:2 * j + 2, :],
                                start=(j == 0), stop=(j == HT // 2 - 1),
                                perf_mode=DR)
                        nc.vector.scalar_tensor_tensor(
                            out=ot_out[:, ot, :], in0=ps, scalar=scv_t[:, :],
                            in1=sr2s[:, ot, :], op0=ALU.mult, op1=ALU.mult)
                        nc.vector.tensor_add(ot_out[:, ot, :], ot_out[:, ot, :],
                                             _f(x2s[:, ot, :]))
                    nc.sync.dma_start(
                        out=out_full[:, t0:t0 + TSD]
                        .rearrange("(ct p) t -> p ct t", p=P),
                        in_=ot_out)
        else:
          with tc.tile_pool(name="wCD", bufs=1) as wCD, \
             tc.tile_pool(name="sCD_strm", bufs=2) as strm, \
             tc.tile_pool(name="sCD_kk", bufs=1) as kkp, \
             tc.tile_pool(name="sCD_work", bufs=3) as work, \
             tc.tile_pool(name="sCD_ps1", bufs=4, space="PSUM") as psC, \
             tc.tile_pool(name="sCD_ps2", bufs=4, space="PSUM") as psD:
            wck_sb = wCD.tile([P, CT, HID], BF16, name="wck_sb")
            _dma_split(nc, wck_sb, ins["wck"].rearrange("(ct p) h -> p ct h", p=P), CT)
            wcv_sb = wCD.tile([P, HT, C], BF16, name="wcv_sb")
            for s in range(NSD):
                t0 = s * TSD
                xk2s = strm.tile([P, CT, TSD], BF16, name="xk2sC", tag="xk2sC", bufs=1)
                _dma_split(nc, xk2s, xk2_d[:, t0:t0 + TSD]
                           .rearrange("(ct p) t -> p ct t", p=P), CT)
                if s == 0:
                    _dma_split(nc, wcv_sb, ins["wcv"]
                               .rearrange("(ht p) o -> p ht o", p=P), HT)
                kk2sb = kkp.tile([P, HT, TSD], BF16, name="kk2sb", tag="kk2sb", bufs=2)
                for ht in range(HT):
                    ps = psC.tile([P, TSD], F32, name="gpsC", tag="gpsC")
                    for ct in range(CT):
                        nc.tensor.matmul(
                            ps, lhsT=wck_sb[:, ct, ht * P:(ht + 1) * P],
                            rhs=xk2s[:, ct, :],
                            start=(ct == 0), stop=(ct == CT - 1))
                    kkr = work.tile([P, TSD], BF16, name="kkr", tag="kkr", bufs=2)
                    nc.scalar.activation(kkr, ps, AF.Relu,
                                         bias=cck_t[:, ht:ht + 1])
                    nc.gpsimd.tensor_mul(kk2sb[:, ht, :], kkr, kkr)
                sr2s = strm.tile([P, CT, TSD], BF16, name="sr2sD", tag="sr2sD", bufs=1)
                _dma_split(nc, sr2s, sr2_d[:, t0:t0 + TSD]
                           .rearrange("(ct p) t -> p ct t", p=P), CT)
                x2s = strm.tile([P, CT, TSD], MMF, name="x2sD", tag="x2sD", bufs=1)
                _dma_split(nc, x2s, x2_d[:, t0:t0 + TSD]
                           .rearrange("(ct p) t -> p ct t", p=P), CT)
                ot_out = strm.tile([P, CT, TSD], F32, name="ot_out", tag="ot_out", bufs=1)
                for ot in range(CT):
                    ps = psD.tile([P, TSD], F32, name="gpsD", tag="gpsD")
                    for ht in range(HT):
                        nc.tensor.matmul(
                            ps, lhsT=wcv_sb[:, ht, ot * P:(ot + 1) * P],
                            rhs=kk2sb[:, ht, :],
                            start=(ht == 0), stop=(ht == HT - 1))
                    nc.vector.tensor_mul(ot_out[:, ot, :], ps, sr2s[:, ot, :])
                    nc.vector.tensor_add(ot_out[:, ot, :], ot_out[:, ot, :],
                                         _f(x2s[:, ot, :]))
                nc.sync.dma_start(
                    out=out_full[:, t0:t0 + TSD].rearrange("(ct p) t -> p ct t", p=P),
                    in_=ot_out)


# ======================= host side =======================

def prep_shared(inputs, cfg):
    """Host-side preprocessing of the shared (non-x) tensors."""
    f32 = np.float32
    g1 = np.asarray(inputs["ln1_g"], f32)
    b1 = np.asarray(inputs["ln1_b"], f32)
    g2 = np.asarray(inputs["ln2_g"], f32)
    b2 = np.asarray(inputs["ln2_b"], f32)
    Wk = np.asarray(inputs["Wk"], f32)
    Wv = np.asarray(inputs["Wv"], f32)
    Wr = np.asarray(inputs["Wr"], f32)
    Wo = np.asarray(inputs["Wo"], f32)
    Wck = np.asarray(inputs["Wck"], f32)
    Wcr = np.asarray(inputs["Wcr"], f32)
    Wcv = np.asarray(inputs["Wcv"], f32)

    td = np.asarray(inputs["time_decay"], np.float64)
    w = -np.exp(td)
    ew = np.exp(w).astype(f32)
    eu = np.exp(np.asarray(inputs["time_first"], np.float64)).astype(f32)

    def safediv(a, b):
        return np.where(b != 0.0, a / np.where(b == 0.0, 1.0, b), 0.0).astype(f32)

    d = {
        "wk": np.ascontiguousarray((Wk * g1[None, :]).T).astype(ml_dtypes.bfloat16),
        "wv": np.ascontiguousarray((Wv * g1[None, :]).T).astype(ml_dtypes.bfloat16),
        "wr": np.ascontiguousarray((Wr * g1[None, :]).T).astype(ml_dtypes.bfloat16),
        "wo": np.ascontiguousarray(Wo.T).astype(ml_dtypes.bfloat16),
        "wcr": np.ascontiguousarray((Wcr * g2[None, :]).T).astype(ml_dtypes.bfloat16),
        "ck": (Wk @ b1).astype(f32),
        "cv": (Wv @ b1).astype(f32),
        "cr": (Wr @ b1).astype(f32),
        "ccr": (Wcr @ b2).astype(f32),
        "tmk": np.asarray(inputs["tm_k"], f32).reshape(-1),
        "tmv": np.asarray(inputs["tm_v"], f32).reshape(-1),
        "tmr": np.asarray(inputs["tm_r"], f32).reshape(-1),
        "cmk": np.asarray(inputs["cm_k"], f32).reshape(-1),
        "cmr": np.asarray(inputs["cm_r"], f32).reshape(-1),
        "ew": ew,
        "eu": eu,
        "one": np.ones((128,), f32),
        "s01": safediv(-b1, g1),
        "s02": safediv(-b2, g2),
    }
    wckT = np.ascontiguousarray((Wck * g2[None, :]).T)  # [C, HID]
    wcvT = np.ascontiguousarray(Wcv.T)                  # [HID, C]
    if cfg.fp8_cd:
        def p2(a):
            return 2.0 ** np.floor(np.log2(224.0 / max(np.abs(a).max(), 1e-30)))

        s_ck, s_cv = p2(wckT), p2(wcvT)
        e4 = ml_dtypes.float8_e4m3
        d["wck"] = np.clip(wckT * s_ck, -240, 240).astype(e4)
        d["wcv"] = np.clip(wcvT * s_cv, -240, 240).astype(e4)
        d["sck_act"] = np.full((128,), S_KKR / (s_ck * S_XK2), f32)
        d["scv_gate"] = np.full((128,), 1.0 / (s_cv * S_KKR * S_KKR), f32)
        d["cck"] = (S_KKR * (Wck @ b2)).astype(f32)
    else:
        d["wck"] = wckT.astype(ml_dtypes.bfloat16)
        d["wcv"] = wcvT.astype(ml_dtypes.bfloat16)
        d["cck"] = (Wck @ b2).astype(f32)
    return d


def make_nc(cfg):
    nc = bacc.Bacc(name="rwkv_block")
    C, T, HID = cfg.C, cfg.T, cfg.HID
    MMF = F32R if cfg.fp32r else F32
    ins = {}

    def din(name, shape, dt=F32):
        ins[name] = nc.dram_tensor(name, shape, dt, kind="ExternalInput").ap()

    din("xT", [C, T], MMF)
    din("one", [P], MMF)
    for k in ("wk", "wv", "wr", "wcr"):
        din(k, [C, C], BF16)
    din("wo", [C, C], BF16)
    WCD = F8 if cfg.fp8_cd else BF16
    din("wck", [C, HID], WCD)
    din("wcv", [HID, C], WCD)
    for k in ("tmk", "tmv", "tmr", "cmk", "cmr", "ew", "eu",
              "ck", "cv", "cr", "ccr", "s01", "s02"):
        din(k, [C])
    din("cck", [HID])
    if cfg.fp8_cd:
        din("sck_act", [P])
        din("scv_gate", [P])
    out = nc.dram_tensor("out", [C, T], F32, kind="ExternalOutput").ap()

    with tile.TileContext(nc) as tc:
        emit(tc, {"out": out}, ins, cfg)
    nc.compile()
    return nc


_CACHED = {}


def kernel(**inputs) -> np.ndarray:
    from concourse.bass_utils import run_bass_kernel_spmd
    cfg = Cfg()
    B, T, C = cfg.B, cfg.T, cfg.C
    x = np.asarray(inputs["x"], np.float32)
    assert x.shape == (B, T, C)

    if "nc" not in _CACHED:
        _CACHED["nc"] = make_nc(cfg)
    nc = _CACHED["nc"]

    shared = prep_shared(inputs, cfg)
    in_maps = []
    for b in range(B):
        m = dict(shared)
        m["xT"] = np.ascontiguousarray(x[b].T)
        in_maps.append(m)

    res = run_bass_kernel_spmd(nc, in_maps, core_ids=list(range(B)))
    out = np.empty((B, T, C), np.float32)
    for b in range(B):
        out[b] = res.results[b]["out"].T
    return out



# revision 19
# speedup vs baseline: 1.0854x; 1.0854x over previous
"""RWKV-4 block (time-mix + channel-mix) Trainium2 Bass kernel.

Sharding: data-parallel over B across 8 NeuronCores (B=8 -> 1 batch el/core,
zero collectives). Everything on-device is feature-major [C partitions, T free]:
 - time_shift == free-dim offset (no data movement)
 - WKV recurrence == native DVE tensor_tensor_scan (state = d0*state + d1)
 - LN stats via ones-matmul partition reduction + K=1 ones-matmul broadcast
 - LN gain/bias folded into the GEMM weights host-side
GEMMs run as fp32r (full-rate fp32 mode). Channel-mix hidden (relu^2 out) is
spilled to DRAM in bf16 and the Wcv GEMM runs in bf16.
"""

import contextlib

import numpy as np
import ml_dtypes

import concourse.bass as bass
import concourse.bacc as bacc
import concourse.mybir as mybir
import concourse.tile as tile

F32 = mybir.dt.float32
F32R = mybir.dt.float32r
BF16 = mybir.dt.bfloat16
AF = mybir.ActivationFunctionType
ALU = mybir.AluOpType
P = 128
EPS = 1e-5


F8 = mybir.dt.float8e4
DR = mybir.MatmulPerfMode.DoubleRow
S_XK2 = 16.0   # xk2 -> fp8 activation scale
S_KKR = 4.0    # kkr carries 4*relu(k1); kk8 = kkr^2 = 16*kk


class Cfg:
    def __init__(self, B=8, T=2048, C=1024, HID=4096,
                 TS_A=256, TS_C=512, TS_D=512, fp32r=True, fp8_cd=True):
        self.B, self.T, self.C, self.HID = B, T, C, HID
        self.TS_A, self.TS_C, self.TS_D = TS_A, TS_C, TS_D
        self.fp32r = fp32r
        self.fp8_cd = fp8_cd
        assert C % P == 0 and HID % P == 0
        assert T % TS_A == 0 and T % TS_C == 0 and T % TS_D == 0


def _dma_split(nc, out_tile, in_3d, n):
    """Per-subtile DMA: out_tile [P, n, W] <- in_3d [P, n, W] (one dma per mid idx).
    Keeps each PE consumer waiting on a single DMA queue sem."""
    for i in range(n):
        nc.sync.dma_start(out=out_tile[:, i, :], in_=in_3d[:, i, :])


def _f(ap):
    """View an f32r AP as plain f32 for non-matmul engines."""
    return ap.bitcast(F32) if ap.dtype == F32R else ap


def emit(tc, outs, ins, cfg):
    nc = tc.nc
    C, T, HID = cfg.C, cfg.T, cfg.HID
    CT, HT = C // P, HID // P
    TSA, TSC, TSD = cfg.TS_A, cfg.TS_C, cfg.TS_D
    NSA, NSC, NSD = T // TSA, T // TSC, T // TSD

    out_full = outs["out"]          # [C, T] f32
    xT = ins["xT"]                  # [C, T] f32
    MMF = F32R if cfg.fp32r else F32

    ctx = contextlib.ExitStack()
    with ctx:
        constp = ctx.enter_context(tc.tile_pool(name="constp", bufs=1))
        dram = ctx.enter_context(tc.tile_pool(name="dram", bufs=1, space="DRAM"))

        # ---- DRAM scratch ----
        rwkv_d = dram.tile([C, T], BF16, name="rwkv_d")
        x2_d = dram.tile([C, T], MMF, name="x2_d")
        sr2_d = dram.tile([C, T], BF16, name="sr2_d")
        xk2_d = dram.tile([C, T], F8 if cfg.fp8_cd else BF16, name="xk2_d")

        # ---- constants ----
        def load_vec(key, n):
            t = constp.tile([P, n // P], F32, name=f"cv_{key}")
            nc.sync.dma_start(out=t, in_=ins[key].rearrange("(o p) -> p o", p=P))
            return t

        tmk_t = load_vec("tmk", C)
        tmv_t = load_vec("tmv", C)
        tmr_t = load_vec("tmr", C)
        cmk_t = load_vec("cmk", C)
        cmr_t = load_vec("cmr", C)
        eu_t = load_vec("eu", C)
        ew_t = load_vec("ew", C)
        ck_t = load_vec("ck", C)
        cv_t = load_vec("cv", C)
        cr_t = load_vec("cr", C)
        ccr_t = load_vec("ccr", C)
        cck_t = load_vec("cck", HID)
        s01_t = load_vec("s01", C)
        s02_t = load_vec("s02", C)

        ones_t = constp.tile([P, 1], MMF, name="ones_t")
        nc.sync.dma_start(out=ones_t, in_=ins["one"].rearrange("(p a) -> p a", a=1))
        ones_r = constp.tile([1, P], MMF, name="ones_r")
        nc.sync.dma_start(out=ones_r, in_=ins["one"].rearrange("(a p) -> a p", a=1))
        eps_t = constp.tile([P, 1], F32, name="eps_t")
        nc.vector.memset(eps_t, EPS)


        def ln_pre(work, psS, xs, TS, n_ct, tag_sq, tag_bufs=1):
            """Stats for LN over partitions: returns mr2 [1,2,TS] = (mean, rstd)."""
            sq = work.tile([P, n_ct, TS], MMF, name="ln_sq", tag=tag_sq, bufs=tag_bufs)
            nc.scalar.square(sq, _f(xs))
            ps_sum = psS.tile([1, TS], F32, name="ps_sum", tag="ps_sum")
            ps_sq = psS.tile([1, TS], F32, name="ps_sq", tag="ps_sq")
            for ct in range(n_ct):
                nc.tensor.matmul(ps_sum, lhsT=ones_t, rhs=xs[:, ct, :],
                                 start=(ct == 0), stop=(ct == n_ct - 1))
            for ct in range(n_ct):
                nc.tensor.matmul(ps_sq, lhsT=ones_t, rhs=sq[:, ct, :],
                                 start=(ct == 0), stop=(ct == n_ct - 1))
            mr2 = work.tile([1, 2, TS], MMF, name="ln_mr2", tag="ln_mr2", bufs=2)
            mrow = mr2[:, 0, :]
            with nc.allow_low_precision(reason="f32r mean for broadcast matmul"):
                nc.vector.tensor_scalar_mul(mrow, ps_sum, 1.0 / C)
            msq = work.tile([1, TS], F32, name="ln_msq", tag="ln_msq")
            nc.scalar.mul(msq, ps_sq, 1.0 / C)
            var = work.tile([1, TS], F32, name="ln_var", tag="ln_var")
            nc.vector.tensor_mul(var, _f(mrow), _f(mrow))
            nc.vector.tensor_sub(var, msq, var)
            sd = work.tile([1, TS], F32, name="ln_sd", tag="ln_sd")
            nc.scalar.activation(sd, var, AF.Sqrt, bias=eps_t[:1, :])
            with nc.allow_low_precision(reason="f32r rstd for broadcast matmul"):
                nc.vector.reciprocal(mr2[:, 1, :], sd)
            return mr2

        def ln_bc(work, psS, mr2, TS):
            """Broadcast (mean, rstd) to 128 partitions via K=1 ones-matmul."""
            bc_ps = psS.tile([P, 2, TS], F32, name="ln_bcps", tag="ln_bcps")
            nc.tensor.matmul(bc_ps.rearrange("p a t -> p (a t)"),
                             lhsT=ones_r,
                             rhs=mr2.rearrange("p a t -> p (a t)"),
                             start=True, stop=True)
            mbc2 = work.tile([P, 2, TS], F32, name="ln_mbc2", tag="ln_mbc2", bufs=2)
            nc.scalar.copy(mbc2, bc_ps)
            return mbc2[:, 0, :], mbc2[:, 1, :]

        # ============== stages A+B share the carried-state pool ==============
        with tc.tile_pool(name="persAB", bufs=1) as persAB:
            # decay broadcast tiles for the scans (same every slice)
            ewb = persAB.tile([P, CT, TSA], F32, name="ewb")
            for ct in range(CT):
                nc.gpsimd.tensor_copy(
                    out=ewb[:, ct, :],
                    in_=ew_t[:, ct:ct + 1].to_broadcast((P, TSA)),
                )
            eub = persAB.tile([P, CT, TSA], BF16, name="eub")
            for ct in range(CT):
                nc.gpsimd.tensor_copy(
                    out=eub[:, ct, :],
                    in_=eu_t[:, ct:ct + 1].to_broadcast((P, TSA)),
                )
            hbuf = persAB.tile([P, CT, TSA + 1], BF16, name="hbuf")
            nc.vector.tensor_copy(out=hbuf[:, :, 0], in_=s01_t)
            stA = persAB.tile([P, CT, 1], F32, name="stA")
            stB = persAB.tile([P, CT, 1], F32, name="stB")
            nc.vector.memset(stA, 0.0)
            nc.vector.memset(stB, 0.0)

            # ================= stage A: time-mix =================
            with tc.tile_pool(name="wA", bufs=1) as wA, \
                 tc.tile_pool(name="sA_strm", bufs=2) as strm, \
                 tc.tile_pool(name="sA_work", bufs=1) as work, \
                 tc.tile_pool(name="sA_ps", bufs=5, space="PSUM") as psA, \
                 tc.tile_pool(name="sA_psS", bufs=1, space="PSUM") as psS:

                wk_sb = wA.tile([P, CT, C], BF16, name="wk_sb")
                wv_sb = wA.tile([P, CT, C], BF16, name="wv_sb")
                wr_sb = wA.tile([P, CT, C], BF16, name="wr_sb")

                def _gemm_a(wsb, mixj, dst, act, cst):
                    for ot in range(CT):
                        ps = psA.tile([P, TSA], F32, name="gps", tag="gps")
                        for ct in range(CT):
                            nc.tensor.matmul(
                                ps, lhsT=wsb[:, ct, ot * P:(ot + 1) * P],
                                rhs=mixj[:, ct, :],
                                start=(ct == 0), stop=(ct == CT - 1))
                        nc.scalar.activation(dst[:, ot, :], ps, act,
                                             bias=cst[:, ot:ot + 1])

                prev = None
                for s in range(NSA + 1):
                    if s < NSA:
                        t0 = s * TSA
                        xs = strm.tile([P, CT, TSA], MMF, name="xs", tag="xs")
                        _dma_split(nc, xs, xT[:, t0:t0 + TSA]
                                   .rearrange("(ct p) t -> p ct t", p=P), CT)
                        mr2 = ln_pre(work, psS, xs, TSA, CT, "scrA", 2)
                        if s == 0:
                            _dma_split(nc, wk_sb, ins["wk"]
                                       .rearrange("(ct p) o -> p ct o", p=P), CT)
                            _dma_split(nc, wv_sb, ins["wv"]
                                       .rearrange("(ct p) o -> p ct o", p=P), CT)
                            _dma_split(nc, wr_sb, ins["wr"]
                                       .rearrange("(ct p) o -> p ct o", p=P), CT)
                    if prev is not None:
                        Kt = work.tile([P, CT, TSA], BF16, name="Kt", tag="Kt",
                                       bufs=2)
                        _gemm_a(wk_sb, prev["mix3"][:, 0], Kt, AF.Exp, ck_t)
                    if s < NSA:
                        m_bc, rs_bc = ln_bc(work, psS, mr2, TSA)
                    if prev is not None:
                        Vt = work.tile([P, CT, TSA], BF16, name="Vt", tag="Vt",
                                       bufs=2)
                        _gemm_a(wv_sb, prev["mix3"][:, 1], Vt, AF.Identity, cv_t)
                        SRt = strm.tile([P, CT, TSA], BF16, name="SRt", tag="SRt",
                                        bufs=2)
                        _gemm_a(wr_sb, prev["mix3"][:, 2], SRt, AF.Sigmoid, cr_t)

                        KV = work.tile([P, CT, TSA], BF16, name="KV", tag="KV",
                                       bufs=2)
                        nc.gpsimd.tensor_mul(KV, Kt, Vt)
                        # scan outputs carry the inbound state in column 0
                        SA = work.tile([P, CT, TSA + 1], F32, name="SA", tag="SA")
                        SB = work.tile([P, CT, TSA + 1], F32, name="SB", tag="scrA",
                                       bufs=2)
                        nc.vector.tensor_copy(out=SA[:, :, 0:1], in_=stA)
                        nc.vector.tensor_copy(out=SB[:, :, 0:1], in_=stB)
                        for ct in range(CT):
                            nc.vector.tensor_tensor_scan(
                                out=SA[:, ct, 1:], data0=ewb[:, ct, :],
                                data1=KV[:, ct, :],
                                initial=stA[:, ct, :], op0=ALU.mult, op1=ALU.add)
                        for ct in range(CT):
                            nc.vector.tensor_tensor_scan(
                                out=SB[:, ct, 1:], data0=ewb[:, ct, :],
                                data1=Kt[:, ct, :],
                                initial=stB[:, ct, :], op0=ALU.mult, op1=ALU.add)
                        # N into KV, D into Kt (in place)
                        Dt = work.tile([P, CT, TSA], F32, name="Dt", tag="rec")
                        for ct in range(CT):
                            eus = eu_t[:, ct:ct + 1]
                            nc.vector.scalar_tensor_tensor(
                                out=KV[:, ct, :], in0=KV[:, ct, :], scalar=eus,
                                in1=SA[:, ct, :TSA], op0=ALU.mult, op1=ALU.add)
                            nc.vector.scalar_tensor_tensor(
                                out=Dt[:, ct, :], in0=Kt[:, ct, :], scalar=eus,
                                in1=SB[:, ct, :TSA], op0=ALU.mult, op1=ALU.add)
                        nc.vector.tensor_copy(out=stA, in_=SA[:, :, TSA:TSA + 1])
                        nc.vector.tensor_copy(out=stB, in_=SB[:, :, TSA:TSA + 1])
                        # y = N / D ; rwkv = y * sigmoid(r) -> bf16 for the spill
                        rec = Dt
                        nc.vector.reciprocal_approx_fast(rec, Dt)
                        rwkv16 = work.tile([P, CT, TSA], BF16, name="rwkv16",
                                           tag="dmixA", bufs=2)
                        nc.gpsimd.tensor_mul(KV, KV, rec)
                        nc.gpsimd.tensor_mul(rwkv16, KV, SRt)
                        nc.sync.dma_start(
                            out=rwkv_d[:, prev["t0"]:prev["t0"] + TSA]
                            .rearrange("(ct p) t -> p ct t", p=P),
                            in_=rwkv16)
                        prev = None
                    if s < NSA:
                        if s > 0:
                            nc.vector.tensor_copy(out=hbuf[:, :, 0:1],
                                                  in_=hbuf[:, :, TSA:TSA + 1])
                        d = work.tile([P, CT, TSA], BF16, name="dmix", tag="dmixA",
                                      bufs=2)
                        for ct in range(CT):
                            xc = work.tile([P, TSA], F32, name="xc", tag="xcA",
                                           bufs=2)
                            nc.vector.tensor_sub(xc, _f(xs[:, ct, :]), m_bc)
                            nc.vector.tensor_mul(hbuf[:, ct, 1:], xc, rs_bc)
                        nc.gpsimd.tensor_sub(d, hbuf[:, :, 1:],
                                             hbuf[:, :, :TSA])
                        mix3 = strm.tile([P, 3, CT, TSA], BF16, name="mix3",
                                         tag="mix3")
                        for j, tmx in enumerate((tmk_t, tmv_t, tmr_t)):
                            for ct in range(CT):
                                nc.vector.scalar_tensor_tensor(
                                    out=mix3[:, j, ct, :], in0=d[:, ct, :],
                                    scalar=tmx[:, ct:ct + 1],
                                    in1=hbuf[:, ct, :TSA],
                                    op0=ALU.mult, op1=ALU.add)
                        prev = {"mix3": mix3, "t0": t0}

            # ======== stage B: Wo + residual, LN2, xk2 spill, Wcr ========
            hbuf2 = persAB.tile([P, CT, TSA + 1], BF16, name="hbuf2")
            nc.vector.tensor_copy(out=hbuf2[:, :, 0], in_=s02_t)

            with tc.tile_pool(name="wB", bufs=1) as wB, \
                 tc.tile_pool(name="sB_strm", bufs=2) as strm, \
                 tc.tile_pool(name="sB_work", bufs=1) as work, \
                 tc.tile_pool(name="sB_ps", bufs=5, space="PSUM") as psB, \
                 tc.tile_pool(name="sB_psS", bufs=1, space="PSUM") as psS:

                wo_sb = wB.tile([P, CT, C], BF16, name="wo_sb")
                wcr_sb = wB.tile([P, CT, C], BF16, name="wcr_sb")

                def _gemm_b(wsb, rhs3, post):
                    for ot in range(CT):
                        ps = psB.tile([P, TSA], F32, name="gpsB", tag="gpsB")
                        for ct in range(CT):
                            nc.tensor.matmul(
                                ps, lhsT=wsb[:, ct, ot * P:(ot + 1) * P],
                                rhs=rhs3[:, ct, :],
                                start=(ct == 0), stop=(ct == CT - 1))
                        post(ot, ps)

                from collections import deque
                pend = deque()
                for s in range(NSA + 2):
                    if s < NSA:
                        t0 = s * TSA
                        rw = strm.tile([P, CT, TSA], BF16, name="rw", tag="rw")
                        _dma_split(nc, rw, rwkv_d[:, t0:t0 + TSA]
                                   .rearrange("(ct p) t -> p ct t", p=P), CT)
                        xs = strm.tile([P, CT, TSA], MMF, name="xsB", tag="xsB")
                        nc.sync.dma_start(out=xs, in_=xT[:, t0:t0 + TSA]
                                          .rearrange("(ct p) t -> p ct t", p=P))
                        if s == 0:
                            _dma_split(nc, wo_sb, ins["wo"]
                                       .rearrange("(ct p) o -> p ct o", p=P), CT)
                            _dma_split(nc, wcr_sb, ins["wcr"]
                                       .rearrange("(ct p) o -> p ct o", p=P), CT)
                        x2s = strm.tile([P, CT, TSA], MMF, name="x2s", tag="x2s")
                        _gemm_b(wo_sb, rw, lambda ot, ps: nc.vector.tensor_add(
                            x2s[:, ot, :], ps, _f(xs[:, ot, :])))
                        nc.sync.dma_start(
                            out=x2_d[:, t0:t0 + TSA]
                            .rearrange("(ct p) t -> p ct t", p=P),
                            in_=x2s)
                    prevb = pend.popleft() if (len(pend) >= 2 or s >= NSA) and pend else None
                    if prevb is not None:
                        sr2s = strm.tile([P, CT, TSA], BF16, name="sr2s", tag="sr2s")
                        _gemm_b(wcr_sb, prevb["mixr"],
                                lambda ot, ps: nc.scalar.activation(
                                    sr2s[:, ot, :], ps, AF.Sigmoid,
                                    bias=ccr_t[:, ot:ot + 1]))
                        nc.sync.dma_start(
                            out=sr2_d[:, prevb["t0"]:prevb["t0"] + TSA]
                            .rearrange("(ct p) t -> p ct t", p=P),
                            in_=sr2s)
                    if s < NSA:
                        mr2 = ln_pre(work, psS, x2s, TSA, CT, "sqB")
                        m_bc, rs_bc = ln_bc(work, psS, mr2, TSA)
                        if s > 0:
                            nc.vector.tensor_copy(out=hbuf2[:, :, 0:1],
                                                  in_=hbuf2[:, :, TSA:TSA + 1])
                        d2 = work.tile([P, CT, TSA], BF16, name="d2", tag="d2",
                                       bufs=2)
                        for ct in range(CT):
                            tmp = work.tile([P, TSA], F32, name="tmpB", tag="tmpB",
                                            bufs=2)
                            nc.vector.tensor_sub(tmp, _f(x2s[:, ct, :]), m_bc)
                            nc.vector.tensor_mul(hbuf2[:, ct, 1:], tmp, rs_bc)
                        nc.gpsimd.tensor_sub(d2, hbuf2[:, :, 1:],
                                             hbuf2[:, :, :TSA])
                        xk2s = work.tile([P, CT, TSA], BF16, name="xk2s",
                                         tag="xk2s", bufs=2)
                        for ct in range(CT):
                            nc.vector.scalar_tensor_tensor(
                                out=xk2s[:, ct, :], in0=d2[:, ct, :],
                                scalar=cmk_t[:, ct:ct + 1], in1=hbuf2[:, ct, :TSA],
                                op0=ALU.mult, op1=ALU.add)
                        if cfg.fp8_cd:
                            xk2q = work.tile([P, CT, TSA], F8, name="xk2q",
                                             tag="xk2q", bufs=2)
                            nc.scalar.activation(xk2q, xk2s, AF.Identity,
                                                 scale=S_XK2)
                            nc.sync.dma_start(
                                out=xk2_d[:, t0:t0 + TSA]
                                .rearrange("(ct p) t -> p ct t", p=P),
                                in_=xk2q)
                        else:
                            nc.sync.dma_start(
                                out=xk2_d[:, t0:t0 + TSA]
                                .rearrange("(ct p) t -> p ct t", p=P),
                                in_=xk2s)
                        mixr = work.tile([P, CT, TSA], BF16, name="mixr",
                                         tag="mixr", bufs=3)
                        for ct in range(CT):
                            nc.vector.scalar_tensor_tensor(
                                out=mixr[:, ct, :], in0=d2[:, ct, :],
                                scalar=cmr_t[:, ct:ct + 1], in1=hbuf2[:, ct, :TSA],
                                op0=ALU.mult, op1=ALU.add)
                        pend.append({"mixr": mixr, "t0": t0})

        # ============ stage CD: Wck + relu^2 (SBUF) + Wcv + gate + residual ============
        if cfg.fp8_cd:
            with tc.tile_pool(name="wCD", bufs=1) as wCD, \
                 tc.tile_pool(name="sCD_strm", bufs=1) as strm, \
                 tc.tile_pool(name="sCD_kk", bufs=2) as kkp, \
                 tc.tile_pool(name="sCD_work", bufs=2) as work, \
                 tc.tile_pool(name="sCD_ps1", bufs=2, space="PSUM") as psC, \
                 tc.tile_pool(name="sCD_ps2", bufs=2, space="PSUM") as psD:
                wck_sb = wCD.tile([P, CT, HID], F8, name="wck_sb")
                _dma_split(nc, wck_sb,
                           ins["wck"].rearrange("(ct p) h -> p ct h", p=P), CT)
                wcv_sb = wCD.tile([P, HT, C], F8, name="wcv_sb")
                sck_t = wCD.tile([P, 1], F32, name="sck_t")
                nc.sync.dma_start(out=sck_t,
                                  in_=ins["sck_act"].rearrange("(p a) -> p a", a=1))
                scv_t = wCD.tile([P, 1], F32, name="scv_t")
                nc.sync.dma_start(out=scv_t,
                                  in_=ins["scv_gate"].rearrange("(p a) -> p a", a=1))
                for s in range(NSD):
                    t0 = s * TSD
                    xk2s = strm.tile([P, CT, TSD], F8, name="xk2sC",
                                     tag="xk2sC", bufs=2)
                    _dma_split(nc, xk2s, xk2_d[:, t0:t0 + TSD]
                               .rearrange("(ct p) t -> p ct t", p=P), CT)
                    if s == 0:
                        _dma_split(nc, wcv_sb, ins["wcv"]
                                   .rearrange("(ht p) o -> p ht o", p=P), HT)
                    kk8 = kkp.tile([P, HT, TSD], F8, name="kk8", tag="kk8")
                    for hh in range(HT // 2):
                        ps = psC.tile([P, 2, TSD], F32, name="gpsC", tag="gpsC")
                        for i in range(2):
                            ht = 2 * hh + i
                            for j in range(CT // 2):
                                nc.tensor.matmul(
                                    ps[:, i, :],
                                    lhsT=wck_sb[:, 2 * j:2 * j + 2,
                                                ht * P:(ht + 1) * P],
                                    rhs=xk2s[:, 2 * j:2 * j + 2, :],
                                    start=(j == 0), stop=(j == CT // 2 - 1),
                                    perf_mode=DR)
                        kkr = work.tile([P, 2, TSD], BF16, name="kkr", tag="kkr")
                        for i in range(2):
                            ht = 2 * hh + i
                            nc.scalar.activation(kkr[:, i, :], ps[:, i, :],
                                                 AF.Relu, scale=sck_t[:, :],
                                                 bias=cck_t[:, ht:ht + 1])
                        nc.scalar.activation(kk8[:, 2 * hh:2 * hh + 2, :], kkr,
                                             AF.Square)
                    sr2s = strm.tile([P, CT, TSD], BF16, name="sr2sD",
                                     tag="sr2sD", bufs=1)
                    _dma_split(nc, sr2s, sr2_d[:, t0:t0 + TSD]
                               .rearrange("(ct p) t -> p ct t", p=P), CT)
                    x2s = strm.tile([P, CT, TSD], MMF, name="x2sD",
                                    tag="x2sD", bufs=1)
                    _dma_split(nc, x2s, x2_d[:, t0:t0 + TSD]
                               .rearrange("(ct p) t -> p ct t", p=P), CT)
                    ot_out = strm.tile([P, CT, TSD], F32, name="ot_out",
                                       tag="ot_out", bufs=1)
                    for ot in range(CT):
                        ps = psD.tile([P, TSD], F32, name="gpsD", tag="gpsD")
                        for j in range(HT // 2):
                            nc.tensor.matmul(
                                ps,
                                lhsT=wcv_sb[:, 2 * j:2 * j + 2,
                                            ot * P:(ot + 1) * P],
                                rhs=kk8[:, 2 * j:2 * j + 2, :],
                                start=(j == 0), stop=(j == HT // 2 - 1),
                                perf_mode=DR)
                        nc.vector.scalar_tensor_tensor(
                            out=ot_out[:, ot, :], in0=ps, scalar=scv_t[:, :],
                            in1=sr2s[:, ot, :], op0=ALU.mult, op1=ALU.mult)
                        nc.vector.tensor_add(ot_out[:, ot, :], ot_out[:, ot, :],
                                             _f(x2s[:, ot, :]))
                    nc.sync.dma_start(
                        out=out_full[:, t0:t0 + TSD]
                        .rearrange("(ct p) t -> p ct t", p=P),
                        in_=ot_out)
        else:
          with tc.tile_pool(name="wCD", bufs=1) as wCD, \
             tc.tile_pool(name="sCD_strm", bufs=2) as strm, \
             tc.tile_pool(name="sCD_kk", bufs=1) as kkp, \
             tc.tile_pool(name="sCD_work", bufs=3) as work, \
             tc.tile_pool(name="sCD_ps1", bufs=4, space="PSUM") as psC, \
             tc.tile_pool(name="sCD_ps2", bufs=4, space="PSUM") as psD:
            wck_sb = wCD.tile([P, CT, HID], BF16, name="wck_sb")
            _dma_split(nc, wck_sb, ins["wck"].rearrange("(ct p) h -> p ct h", p=P), CT)
            wcv_sb = wCD.tile([P, HT, C], BF16, name="wcv_sb")
            for s in range(NSD):
                t0 = s * TSD
                xk2s = strm.tile([P, CT, TSD], BF16, name="xk2sC", tag="xk2sC", bufs=1)
                _dma_split(nc, xk2s, xk2_d[:, t0:t0 + TSD]
                           .rearrange("(ct p) t -> p ct t", p=P), CT)
                if s == 0:
                    _dma_split(nc, wcv_sb, ins["wcv"]
                               .rearrange("(ht p) o -> p ht o", p=P), HT)
                kk2sb = kkp.tile([P, HT, TSD], BF16, name="kk2sb", tag="kk2sb", bufs=2)
                for ht in range(HT):
                    ps = psC.tile([P, TSD], F32, name="gpsC", tag="gpsC")
                    for ct in range(CT):
                        nc.tensor.matmul(
                            ps, lhsT=wck_sb[:, ct, ht * P:(ht + 1) * P],
                            rhs=xk2s[:, ct, :],
                            start=(ct == 0), stop=(ct == CT - 1))
                    kkr = work.tile([P, TSD], BF16, name="kkr", tag="kkr", bufs=2)
                    nc.scalar.activation(kkr, ps, AF.Relu,
                                         bias=cck_t[:, ht:ht + 1])
                    nc.gpsimd.tensor_mul(kk2sb[:, ht, :], kkr, kkr)
                sr2s = strm.tile([P, CT, TSD], BF16, name="sr2sD", tag="sr2sD", bufs=1)
                _dma_split(nc, sr2s, sr2_d[:, t0:t0 + TSD]
                           .rearrange("(ct p) t -> p ct t", p=P), CT)
                x2s = strm.tile([P, CT, TSD], MMF, name="x2sD", tag="x2sD", bufs=1)
                _dma_split(nc, x2s, x2_d[:, t0:t0 + TSD]
                           .rearrange("(ct p) t -> p ct t", p=P), CT)
                ot_out = strm.tile([P, CT, TSD], F32, name="ot_out", tag="ot_out", bufs=1)
                for ot in range(CT):
                    ps = psD.tile([P, TSD], F32, name="gpsD", tag="gpsD")
                    for ht in range(HT):
                        nc.tensor.matmul(
                            ps, lhsT=wcv_sb[:, ht, ot * P:(ot + 1) * P],
                            rhs=kk2sb[:, ht, :],
                            start=(ht == 0), stop=(ht == HT - 1))
                    nc.vector.tensor_mul(ot_out[:, ot, :], ps, sr2s[:, ot, :])
                    nc.vector.tensor_add(ot_out[:, ot, :], ot_out[:, ot, :],
                                         _f(x2s[:, ot, :]))
                nc.sync.dma_start(
                    out=out_full[:, t0:t0 + TSD].rearrange("(ct p) t -> p ct t", p=P),
                    in_=ot_out)


# ======================= host side =======================

def prep_shared(inputs, cfg):
    """Host-side preprocessing of the shared (non-x) tensors."""
    f32 = np.float32
    g1 = np.asarray(inputs["ln1_g"], f32)
    b1 = np.asarray(inputs["ln1_b"], f32)
    g2 = np.asarray(inputs["ln2_g"], f32)
    b2 = np.asarray(inputs["ln2_b"], f32)
    Wk = np.asarray(inputs["Wk"], f32)
    Wv = np.asarray(inputs["Wv"], f32)
    Wr = np.asarray(inputs["Wr"], f32)
    Wo = np.asarray(inputs["Wo"], f32)
    Wck = np.asarray(inputs["Wck"], f32)
    Wcr = np.asarray(inputs["Wcr"], f32)
    Wcv = np.asarray(inputs["Wcv"], f32)

    td = np.asarray(inputs["time_decay"], np.float64)
    w = -np.exp(td)
    ew = np.exp(w).astype(f32)
    eu = np.exp(np.asarray(inputs["time_first"], np.float64)).astype(f32)

    def safediv(a, b):
        return np.where(b != 0.0, a / np.where(b == 0.0, 1.0, b), 0.0).astype(f32)

    d = {
        "wk": np.ascontiguousarray((Wk * g1[None, :]).T).astype(ml_dtypes.bfloat16),
        "wv": np.ascontiguousarray((Wv * g1[None, :]).T).astype(ml_dtypes.bfloat16),
        "wr": np.ascontiguousarray((Wr * g1[None, :]).T).astype(ml_dtypes.bfloat16),
        "wo": np.ascontiguousarray(Wo.T).astype(ml_dtypes.bfloat16),
        "wcr": np.ascontiguousarray((Wcr * g2[None, :]).T).astype(ml_dtypes.bfloat16),
        "ck": (Wk @ b1).astype(f32),
        "cv": (Wv @ b1).astype(f32),
        "cr": (Wr @ b1).astype(f32),
        "ccr": (Wcr @ b2).astype(f32),
        "tmk": np.asarray(inputs["tm_k"], f32).reshape(-1),
        "tmv": np.asarray(inputs["tm_v"], f32).reshape(-1),
        "tmr": np.asarray(inputs["tm_r"], f32).reshape(-1),
        "cmk": np.asarray(inputs["cm_k"], f32).reshape(-1),
        "cmr": np.asarray(inputs["cm_r"], f32).reshape(-1),
        "ew": ew,
        "eu": eu,
        "one": np.ones((128,), f32),
        "s01": safediv(-b1, g1),
        "s02": safediv(-b2, g2),
    }
    wckT = np.ascontiguousarray((Wck * g2[None, :]).T)  # [C, HID]
    wcvT = np.ascontiguousarray(Wcv.T)                  # [HID, C]
    if cfg.fp8_cd:
        def p2(a):
            return 2.0 ** np.floor(np.log2(224.0 / max(np.abs(a).max(), 1e-30)))

        s_ck, s_cv = p2(wckT), p2(wcvT)
        e4 = ml_dtypes.float8_e4m3
        d["wck"] = np.clip(wckT * s_ck, -240, 240).astype(e4)
        d["wcv"] = np.clip(wcvT * s_cv, -240, 240).astype(e4)
        d["sck_act"] = np.full((128,), S_KKR / (s_ck * S_XK2), f32)
        d["scv_gate"] = np.full((128,), 1.0 / (s_cv * S_KKR * S_KKR), f32)
        d["cck"] = (S_KKR * (Wck @ b2)).astype(f32)
    else:
        d["wck"] = wckT.astype(ml_dtypes.bfloat16)
        d["wcv"] = wcvT.astype(ml_dtypes.bfloat16)
        d["cck"] = (Wck @ b2).astype(f32)
    return d


def make_nc(cfg):
    nc = bacc.Bacc(name="rwkv_block")
    C, T, HID = cfg.C, cfg.T, cfg.HID
    MMF = F32R if cfg.fp32r else F32
    ins = {}

    def din(name, shape, dt=F32):
        ins[name] = nc.dram_tensor(name, shape, dt, kind="ExternalInput").ap()

    din("xT", [C, T], MMF)
    din("one", [P], MMF)
    for k in ("wk", "wv", "wr", "wcr"):
        din(k, [C, C], BF16)
    din("wo", [C, C], BF16)
    WCD = F8 if cfg.fp8_cd else BF16
    din("wck", [C, HID], WCD)
    din("wcv", [HID, C], WCD)
    for k in ("tmk", "tmv", "tmr", "cmk", "cmr", "ew", "eu",
              "ck", "cv", "cr", "ccr", "s01", "s02"):
        din(k, [C])
    din("cck", [HID])
    if cfg.fp8_cd:
        din("sck_act", [P])
        din("scv_gate", [P])
    out = nc.dram_tensor("out", [C, T], F32, kind="ExternalOutput").ap()

    with tile.TileContext(nc) as tc:
        emit(tc, {"out": out}, ins, cfg)
    nc.compile()
    return nc


_CACHED = {}


def kernel(**inputs) -> np.ndarray:
    from concourse.bass_utils import run_bass_kernel_spmd
    cfg = Cfg()
    B, T, C = cfg.B, cfg.T, cfg.C
    x = np.asarray(inputs["x"], np.float32)
    assert x.shape == (B, T, C)

    if "nc" not in _CACHED:
        _CACHED["nc"] = make_nc(cfg)
    nc = _CACHED["nc"]

    shared = prep_shared(inputs, cfg)
    in_maps = []
    for b in range(B):
        m = dict(shared)
        m["xT"] = np.ascontiguousarray(x[b].T)
        in_maps.append(m)

    res = run_bass_kernel_spmd(nc, in_maps, core_ids=list(range(B)))
    out = np.empty((B, T, C), np.float32)
    for b in range(B):
        out[b] = res.results[b]["out"].T
    return out

